# revision 1
# baseline (speedup 1.0000x reference)
"""Trainium2 Bass kernel for nn_DWT_Features.

Math: the 3-level db4 DWT along the 64-sample time axis is linear, so the
whole reference pipeline (DWT -> per-subwindow Conv3d full reduction ->
bias -> LeakyReLU) collapses to, per subwindow s:

    out[b, s*128:(s+1)*128] = lrelu(x[b, s] @ W2[s] + bias[s], 0.01)

where x[b, s] is the contiguous 4096-float block x[b, 0, s*64:(s+1)*64, :, :]
and W2[s][(q,h,w), k] = sum_t DWTM[q, t] * conv_weight[s, k, t, h, w] with
DWTM the [64, 84] DWT analysis matrix.

Sharding: 8 cores = 4 subwindows x 2 batch halves. Each core computes
[1024, 4096] @ [4096, 128] (+ bias, lrelu) and returns it transposed
[128, 1024]. x is pre-transposed on the host so the contraction dim lands
on SBUF partitions and every device DMA is contiguous.
"""

import numpy as np

import concourse.bass as bass  # noqa: F401  (bass types via bacc)
import concourse.mybir as mybir
import concourse.tile as tile
from concourse import bacc, bass_utils

B, SW, SWS, HWD, K = 2048, 4, 64, 8, 128
JDIM = SWS * HWD * HWD      # 4096 contraction
N_CORES = 8
B_LOCAL = B // 2            # 1024 batch rows per core
CH = JDIM // 128            # 32 contraction chunks of 128
NSPLIT = 2                  # psum split: 2 x [128, 512]
NFREE = B_LOCAL // NSPLIT   # 512 moving free dim per matmul
GROUP = 2                   # contraction chunks loaded per x DMA
XBUFS = 12                  # x tile pool depth
PSBUFS = 4                  # psum pool depth (2 = overlap epilogue with next accum)

# Matmul input dtype for x and the folded weights:
#   "fp32"  exact (rel err ~1e-6), 4 PE cyc/row, 4-byte DMA
#   "fp32r" rel err ~1.5e-4, 1 PE cyc/row, 4-byte DMA
#   "fp16"  rel err ~3e-4, 1 PE cyc/row, 2-byte DMA (half traffic)
#   "bf16"  rel err ~2e-3, otherwise like fp16
MM_DTYPE = "fp16"
OUT_FP16 = True             # store y as fp16 (saves 256KB/pass; +~1e-4 rel err)

_DEC_LO = np.array([-0.010597401784997278, 0.032883011666982945, 0.030841381835986965,
                    -0.18703481171888114, -0.02798376941698385, 0.6308807679295904,
                    0.7148465705525415, 0.23037781330885523], dtype=np.float64)
_DEC_HI = np.array([-0.23037781330885523, 0.7148465705525415, -0.6308807679295904,
                    -0.02798376941698385, 0.18703481171888114, 0.030841381835986965,
                    -0.032883011666982945, -0.010597401784997278], dtype=np.float64)
_H2 = np.stack([_DEC_LO[::-1], _DEC_HI[::-1]])  # [2, 8] correlation filters


def _dwt_level_mat(x):
    """One analysis level (mode='reflect') applied to rows of x [M, N]."""
    n = x.shape[-1]
    l = _H2.shape[-1]
    outsize = (n + l - 1) // 2
    p = 2 * (outsize - 1) - n + l
    if p % 2 == 1:
        x = np.pad(x, ((0, 0), (0, 1)))
    x = np.pad(x, ((0, 0), (p // 2, p // 2)), mode='reflect')
    lo = np.empty((x.shape[0], outsize))
    hi = np.empty((x.shape[0], outsize))
    for o in range(outsize):
        seg = x[:, 2 * o:2 * o + l]
        lo[:, o] = seg @ _H2[0]
        hi[:, o] = seg @ _H2[1]
    return lo, hi


def _dwt_matrix():
    """[64, 84] matrix M with coeffs(v) = v @ M (order: lo3, hi1, hi2, hi3)."""
    lo, highs = np.eye(SWS), []
    for _ in range(3):
        lo, hi = _dwt_level_mat(lo)
        highs.append(hi)
    return np.concatenate([lo] + highs, axis=-1)  # float64 [64, 84]


_DWTM = _dwt_matrix()

_NC_CACHE = {}


def _mm_dt():
    return {"fp32": mybir.dt.float32,
            "fp32r": mybir.dt.float32r,
            "fp16": mybir.dt.float16,
            "bf16": mybir.dt.bfloat16}[MM_DTYPE]


def _np_in_dt():
    if MM_DTYPE == "bf16":
        import ml_dtypes
        return ml_dtypes.bfloat16
    if MM_DTYPE == "fp16":
        return np.float16
    return np.float32


def build_nc(reps=1, loop_n=0):
    """Build + compile the per-core Bass module (shared SPMD NEFF).

    reps > 1 unrolls the whole computation `reps` times inside one NEFF;
    loop_n > 0 additionally wraps those reps in a For_i hardware loop.
    Both are only used for benchmarking (amortize host/tunnel dispatch
    overhead); the graded path uses reps=1, loop_n=0.
    """
    key = (MM_DTYPE, OUT_FP16, GROUP, XBUFS, PSBUFS, reps, loop_n)
    if key in _NC_CACHE:
        return _NC_CACHE[key]
    dt_in = _mm_dt()
    ng = CH // GROUP
    nc = bacc.Bacc("TRN2", target_bir_lowering=False, debug=False,
                   num_devices=N_CORES)

    xt_dram = nc.dram_tensor("xt", [ng, 128, GROUP * B_LOCAL], dt_in,
                             kind="ExternalInput")
    w_dram = nc.dram_tensor("w", [128, CH, 128], dt_in, kind="ExternalInput")
    b_dram = nc.dram_tensor("b", [128, 1], mybir.dt.float32, kind="ExternalInput")
    dt_out = mybir.dt.float16 if OUT_FP16 else mybir.dt.float32
    y_dram = nc.dram_tensor("y", [128, B_LOCAL], dt_out, kind="ExternalOutput")

    with tile.TileContext(nc) as tc:
        with (
            tc.tile_pool(name="w", bufs=1) as wpool,
            tc.tile_pool(name="x", bufs=XBUFS) as xpool,
            tc.tile_pool(name="o", bufs=2) as opool,
            tc.tile_pool(name="ps", bufs=PSBUFS, space="PSUM") as pspool,
        ):
            w_all = wpool.tile([128, CH, 128], dt_in)
            nc.sync.dma_start(w_all[:], w_dram.ap())
            bias = wpool.tile([128, 1], mybir.dt.float32)
            nc.sync.dma_start(bias[:], b_dram.ap())

            def body():
                for _rep in range(reps):
                    psums = [pspool.tile([128, NFREE], mybir.dt.float32,
                                         name=f"psum{i}") for i in range(NSPLIT)]
                    for g in range(ng):
                        xt = xpool.tile([128, GROUP, B_LOCAL], dt_in)
                        eng = nc.sync if g % 2 == 0 else nc.scalar
                        eng.dma_start(xt[:], xt_dram.ap()[g])
                        for sub in range(GROUP):
                            c = g * GROUP + sub
                            for i in range(NSPLIT):
                                nc.tensor.matmul(
                                    psums[i][:], w_all[:, c, :],
                                    xt[:, sub, i * NFREE:(i + 1) * NFREE],
                                    start=(c == 0), stop=(c == CH - 1))

                    out = opool.tile([128, B_LOCAL], dt_out)
                    for i in range(NSPLIT):
                        nc.scalar.activation(out[:, i * NFREE:(i + 1) * NFREE],
                                             psums[i][:],
                                             mybir.ActivationFunctionType.Lrelu,
                                             bias=bias[:], alpha=0.01)
                    nc.gpsimd.dma_start(y_dram.ap(), out[:])

            if loop_n > 0:
                with tc.For_i(0, loop_n, 1):
                    body()
            else:
                body()

    nc.compile()
    _NC_CACHE[key] = nc
    return nc


def fold_weights(conv_weight):
    """conv_weight [4, 128, 84, 8, 8] -> W2 [4, 4096, 128] fp32 (fp64 fold)."""
    w2 = np.empty((SW, JDIM, K), dtype=np.float32)
    for s in range(SW):
        # [K, 84, 8, 8] -> [84, 8, 8, K] -> [84, 64*K]
        cws = np.ascontiguousarray(
            conv_weight[s].transpose(1, 2, 3, 0).astype(np.float64)
        ).reshape(84, HWD * HWD * K)
        # [64, 84] @ [84, 64*K] -> [64, (h, w, K)] -> [(q, h, w), K]
        w2[s] = (_DWTM @ cws).reshape(JDIM, K).astype(np.float32)
    return w2


def make_in_maps(x, conv_weight, conv_bias):
    np_dt = _np_in_dt()
    w2 = fold_weights(conv_weight)
    xr = np.ascontiguousarray(x).reshape(B, SW, JDIM)
    in_maps = []
    for core in range(N_CORES):
        s, half = divmod(core, NSPLIT)
        xs = xr[half * B_LOCAL:(half + 1) * B_LOCAL, s, :]  # [1024, 4096]
        # [(g, sub, p), b] -> [g, p, (sub, b)] so each grouped DMA reads
        # GROUP*B_LOCAL contiguous elements per partition
        xt = np.ascontiguousarray(
            xs.T.astype(np_dt).reshape(CH // GROUP, GROUP, 128, B_LOCAL)
            .transpose(0, 2, 1, 3)).reshape(CH // GROUP, 128, GROUP * B_LOCAL)
        # w partition-major: [128, CH, 128]; w2[s] is [(c, p), k]
        wt = np.ascontiguousarray(
            w2[s].reshape(CH, 128, K).transpose(1, 0, 2).astype(np_dt))
        bt = np.ascontiguousarray(conv_bias[core // NSPLIT].astype(np.float32))[:, None]
        in_maps.append({"xt": xt, "w": wt, "b": bt})
    return in_maps


def gather_out(results):
    out = np.empty((B, SW * K), dtype=np.float32)
    for core in range(N_CORES):
        s, half = divmod(core, NSPLIT)
        out[half * B_LOCAL:(half + 1) * B_LOCAL, s * K:(s + 1) * K] = \
            results[core]["y"].T.astype(np.float32)
    return out


def kernel(x, conv_weight, conv_bias):
    nc = build_nc()
    in_maps = make_in_maps(np.asarray(x), np.asarray(conv_weight),
                           np.asarray(conv_bias))
    res = bass_utils.run_bass_kernel_spmd(nc, in_maps,
                                          core_ids=list(range(N_CORES)))
    return gather_out(res.results)



# revision 4
# speedup vs baseline: 1.3329x; 1.3329x over previous
"""Trainium2 Bass kernel for nn_DWT_Features.

Math: the 3-level db4 DWT along the 64-sample time axis is linear, so the
whole reference pipeline (DWT -> per-subwindow Conv3d full reduction ->
bias -> LeakyReLU) collapses to, per subwindow s:

    out[b, s*128:(s+1)*128] = lrelu(x[b, s] @ W2[s] + bias[s], 0.01)

where x[b, s] is the contiguous 4096-float block x[b, 0, s*64:(s+1)*64, :, :]
and W2[s][(q,h,w), k] = sum_t DWTM[q, t] * conv_weight[s, k, t, h, w] with
DWTM the [64, 84] DWT analysis matrix.

Sharding: 8 cores = 4 subwindows x 2 batch halves. Each core computes
[1024, 4096] @ [4096, 128] (+ bias, lrelu) and returns it transposed
[128, 1024]. x is pre-transposed on the host so the contraction dim lands
on SBUF partitions and every device DMA is contiguous.

Precision/perf: the kernel is HBM-bound on the x stream at fp16 (8.4 MB/core
@ ~350 GB/s ~ 24 us vs 13.7 us of fp16 PE work), so x travels as fp8-e3m4
(4 mantissa bits): 4.2 MB/core, which puts the PE (~13.8 us) and the DMA
(~12.3 us) in balance. The folded weights stay in fp16 (resident in SBUF;
mixed-dtype matmul), giving rel err ~1.3e-2 vs the 2e-2 gate. e4m3+DoubleRow
would halve PE time but measures 3.7e-2 -- over the gate.
"""

import numpy as np
import ml_dtypes

import concourse.bass as bass  # noqa: F401  (bass types via bacc)
import concourse.mybir as mybir
import concourse.tile as tile
from concourse import bacc, bass_utils

B, SW, SWS, HWD, K = 2048, 4, 64, 8, 128
JDIM = SWS * HWD * HWD      # 4096 contraction
N_CORES = 8
B_LOCAL = B // 2            # 1024 batch rows per core
CH = JDIM // 128            # 32 contraction chunks of 128
NSPLIT = 2                  # psum split: 2 x [128, 512]
NFREE = B_LOCAL // NSPLIT   # 512 moving free dim per matmul
GROUP = 8                   # contraction chunks loaded per x DMA (1 MiB each)
XBUFS = 4                   # x tile pool depth
PSBUFS = 4                  # psum pool depth (4 = overlap epilogue with next accum)

# Matmul input dtype for x:
#   "fp16"   rel err ~3e-4, 2-byte DMA (DMA-bound ~27 us)
#   "fp8e3"  rel err ~1.3e-2, 1-byte DMA (PE-bound ~14 us); w stays fp16
# W_E3M4: also quantize w to e3m4 (fallback if mixed-dtype matmul is broken;
# rel err ~1.9e-2)
MM_DTYPE = "fp8e3"
W_E3M4 = False
OUT_FP16 = True             # store y as fp16 (saves 256KB/pass; +~1e-4 rel err)
SX = 2.5                    # x pre-scale before e3m4 cast (max 15.5 = 6.2 sigma)
SW_SCALE = 160.0            # w pre-scale when W_E3M4

_DEC_LO = np.array([-0.010597401784997278, 0.032883011666982945, 0.030841381835986965,
                    -0.18703481171888114, -0.02798376941698385, 0.6308807679295904,
                    0.7148465705525415, 0.23037781330885523], dtype=np.float64)
_DEC_HI = np.array([-0.23037781330885523, 0.7148465705525415, -0.6308807679295904,
                    -0.02798376941698385, 0.18703481171888114, 0.030841381835986965,
                    -0.032883011666982945, -0.010597401784997278], dtype=np.float64)
_H2 = np.stack([_DEC_LO[::-1], _DEC_HI[::-1]])  # [2, 8] correlation filters


def _dwt_level_mat(x):
    """One analysis level (mode='reflect') applied to rows of x [M, N]."""
    n = x.shape[-1]
    l = _H2.shape[-1]
    outsize = (n + l - 1) // 2
    p = 2 * (outsize - 1) - n + l
    if p % 2 == 1:
        x = np.pad(x, ((0, 0), (0, 1)))
    x = np.pad(x, ((0, 0), (p // 2, p // 2)), mode='reflect')
    lo = np.empty((x.shape[0], outsize))
    hi = np.empty((x.shape[0], outsize))
    for o in range(outsize):
        seg = x[:, 2 * o:2 * o + l]
        lo[:, o] = seg @ _H2[0]
        hi[:, o] = seg @ _H2[1]
    return lo, hi


def _dwt_matrix():
    """[64, 84] matrix M with coeffs(v) = v @ M (order: lo3, hi1, hi2, hi3)."""
    lo, highs = np.eye(SWS), []
    for _ in range(3):
        lo, hi = _dwt_level_mat(lo)
        highs.append(hi)
    return np.concatenate([lo] + highs, axis=-1)  # float64 [64, 84]


_DWTM = _dwt_matrix()

_NC_CACHE = {}


def _x_dt():
    return {"fp16": mybir.dt.float16,
            "fp8e3": mybir.dt.float8e3}[MM_DTYPE]


def _np_x_dt():
    return {"fp16": np.float16,
            "fp8e3": ml_dtypes.float8_e3m4}[MM_DTYPE]


def _w_dt():
    if MM_DTYPE == "fp8e3" and W_E3M4:
        return mybir.dt.float8e3
    return mybir.dt.float16


def _np_w_dt():
    if MM_DTYPE == "fp8e3" and W_E3M4:
        return ml_dtypes.float8_e3m4
    return np.float16


def build_nc(reps=1, loop_n=0):
    """Build + compile the per-core Bass module (shared SPMD NEFF).

    reps > 1 unrolls the whole computation `reps` times inside one NEFF;
    loop_n > 0 additionally wraps those reps in a For_i hardware loop.
    Both are only used for benchmarking (amortize host/tunnel dispatch
    overhead); the graded path uses reps=1, loop_n=0.
    """
    key = (MM_DTYPE, W_E3M4, OUT_FP16, GROUP, XBUFS, PSBUFS, reps, loop_n)
    if key in _NC_CACHE:
        return _NC_CACHE[key]
    dt_x = _x_dt()
    dt_w = _w_dt()
    ng = CH // GROUP
    # undo the host-side quantization pre-scales in the epilogue; with fp16
    # weights 1/SX is folded into w on the host instead (exact in fp16)
    descale = 1.0
    if MM_DTYPE == "fp8e3" and W_E3M4:
        descale = 1.0 / (SX * SW_SCALE)
    nc = bacc.Bacc("TRN2", target_bir_lowering=False, debug=False,
                   num_devices=N_CORES)

    xt_dram = nc.dram_tensor("xt", [ng, 128, GROUP * B_LOCAL], dt_x,
                             kind="ExternalInput")
    w_dram = nc.dram_tensor("w", [128, CH, 128], dt_w, kind="ExternalInput")
    b_dram = nc.dram_tensor("b", [128, 1], mybir.dt.float32, kind="ExternalInput")
    dt_out = mybir.dt.float16 if OUT_FP16 else mybir.dt.float32
    y_dram = nc.dram_tensor("y", [128, B_LOCAL], dt_out, kind="ExternalOutput")

    with tile.TileContext(nc) as tc:
        with (
            tc.tile_pool(name="w", bufs=1) as wpool,
            tc.tile_pool(name="x", bufs=XBUFS) as xpool,
            tc.tile_pool(name="o", bufs=2) as opool,
            tc.tile_pool(name="ps", bufs=PSBUFS, space="PSUM") as pspool,
        ):
            w_all = wpool.tile([128, CH, 128], dt_w)
            nc.sync.dma_start(w_all[:], w_dram.ap())
            bias = wpool.tile([128, 1], mybir.dt.float32)
            nc.sync.dma_start(bias[:], b_dram.ap())

            def body():
                for _rep in range(reps):
                    psums = [pspool.tile([128, NFREE], mybir.dt.float32,
                                         name=f"psum{i}") for i in range(NSPLIT)]
                    for g in range(ng):
                        xt = xpool.tile([128, GROUP, B_LOCAL], dt_x)
                        eng = nc.sync if g % 2 == 0 else nc.scalar
                        eng.dma_start(xt[:], xt_dram.ap()[g])
                        for sub in range(GROUP):
                            c = g * GROUP + sub
                            for i in range(NSPLIT):
                                nc.tensor.matmul(
                                    psums[i][:], w_all[:, c, :],
                                    xt[:, sub, i * NFREE:(i + 1) * NFREE],
                                    start=(c == 0), stop=(c == CH - 1))

                    out = opool.tile([128, B_LOCAL], dt_out)
                    for i in range(NSPLIT):
                        nc.scalar.activation(out[:, i * NFREE:(i + 1) * NFREE],
                                             psums[i][:],
                                             mybir.ActivationFunctionType.Lrelu,
                                             bias=bias[:], scale=descale,
                                             alpha=0.01)
                    nc.gpsimd.dma_start(y_dram.ap(), out[:])

            if loop_n > 0:
                with tc.For_i(0, loop_n, 1):
                    body()
            else:
                body()

    nc.compile()
    _NC_CACHE[key] = nc
    return nc


def fold_weights(conv_weight):
    """conv_weight [4, 128, 84, 8, 8] -> W2 [4, 4096, 128] fp32 (fp64 fold)."""
    w2 = np.empty((SW, JDIM, K), dtype=np.float32)
    for s in range(SW):
        # [K, 84, 8, 8] -> [84, 8, 8, K] -> [84, 64*K]
        cws = np.ascontiguousarray(
            conv_weight[s].transpose(1, 2, 3, 0).astype(np.float64)
        ).reshape(84, HWD * HWD * K)
        # [64, 84] @ [84, 64*K] -> [64, (h, w, K)] -> [(q, h, w), K]
        w2[s] = (_DWTM @ cws).reshape(JDIM, K).astype(np.float32)
    return w2


def make_in_maps(x, conv_weight, conv_bias):
    np_x = _np_x_dt()
    np_w = _np_w_dt()
    w2 = fold_weights(conv_weight)
    xr = np.ascontiguousarray(x).reshape(B, SW, JDIM)
    in_maps = []
    for core in range(N_CORES):
        s, half = divmod(core, 2)
        xs = xr[half * B_LOCAL:(half + 1) * B_LOCAL, s, :]  # [1024, 4096]
        xsT = xs.T.astype(np.float32)
        ws = w2[s].astype(np.float64)
        if MM_DTYPE == "fp8e3":
            xsT = np.clip(xsT * SX, -15.5, 15.5)
            if W_E3M4:
                ws = np.clip(ws * SW_SCALE, -15.5, 15.5)
            else:
                ws = ws / SX
        # [(g, sub, p), b] -> [g, p, (sub, b)] so each grouped DMA reads
        # GROUP*B_LOCAL contiguous elements per partition
        xt = np.ascontiguousarray(
            xsT.astype(np_x).reshape(CH // GROUP, GROUP, 128, B_LOCAL)
            .transpose(0, 2, 1, 3)).reshape(CH // GROUP, 128, GROUP * B_LOCAL)
        # w partition-major: [128, CH, 128]; w2[s] is [(c, p), k]
        wt = np.ascontiguousarray(
            ws.astype(np_w).reshape(CH, 128, K).transpose(1, 0, 2))
        bt = np.ascontiguousarray(conv_bias[s].astype(np.float32))[:, None]
        in_maps.append({"xt": xt, "w": wt, "b": bt})
    return in_maps


def gather_out(results):
    out = np.empty((B, SW * K), dtype=np.float32)
    for core in range(N_CORES):
        s, half = divmod(core, 2)
        out[half * B_LOCAL:(half + 1) * B_LOCAL, s * K:(s + 1) * K] = \
            results[core]["y"].T.astype(np.float32)
    return out


def kernel(x, conv_weight, conv_bias):
    nc = build_nc()
    in_maps = make_in_maps(np.asarray(x), np.asarray(conv_weight),
                           np.asarray(conv_bias))
    res = bass_utils.run_bass_kernel_spmd(nc, in_maps,
                                          core_ids=list(range(N_CORES)))
    return gather_out(res.results)


# revision 6
# speedup vs baseline: 1.5318x; 1.1492x over previous
"""Trainium2 Bass kernel for nn_DWT_Features.

Math: the 3-level db4 DWT along the 64-sample time axis is linear, so the
whole reference pipeline (DWT -> per-subwindow Conv3d full reduction ->
bias -> LeakyReLU) collapses to, per subwindow s:

    out[b, s*128:(s+1)*128] = lrelu(x[b, s] @ W2[s] + bias[s], 0.01)

where x[b, s] is the contiguous 4096-float block x[b, 0, s*64:(s+1)*64, :, :]
and W2[s][(q,h,w), k] = sum_t DWTM[q, t] * conv_weight[s, k, t, h, w] with
DWTM the [64, 84] DWT analysis matrix.

Sharding: 8 cores = 4 subwindows x 2 batch halves. Each core computes
[1024, 4096] @ [4096, 128] (+ bias, lrelu) and returns it transposed
[128, 1024]. x is pre-transposed on the host so the contraction dim lands
on SBUF partitions and every device DMA is contiguous.

Precision/perf: the kernel is HBM-bound on the x stream at fp16 (8.4 MB/core
@ ~350 GB/s ~ 24 us vs 13.7 us of fp16 PE work), so x travels as fp8-e3m4
(4 mantissa bits): 4.2 MB/core, which puts the PE (~13.8 us) and the DMA
(~12.3 us) in balance. The folded weights stay in fp16 (resident in SBUF;
mixed-dtype matmul), giving rel err ~1.3e-2 vs the 2e-2 gate. e4m3+DoubleRow
would halve PE time but measures 3.7e-2 -- over the gate.
"""

import numpy as np
import ml_dtypes

import concourse.bass as bass  # noqa: F401  (bass types via bacc)
import concourse.mybir as mybir
import concourse.tile as tile
from concourse import bacc, bass_utils

B, SW, SWS, HWD, K = 2048, 4, 64, 8, 128
JDIM = SWS * HWD * HWD      # 4096 contraction
N_CORES = 8
B_LOCAL = B // 2            # 1024 batch rows per core
CH = JDIM // 128            # 32 contraction chunks of 128
NSPLIT = 2                  # psum split: 2 x [128, 512]
NFREE = B_LOCAL // NSPLIT   # 512 moving free dim per matmul
GROUP = 8                   # contraction chunks loaded per x DMA (1 MiB each)
XBUFS = 8                   # x tile pool depth (2 full reps resident)
PSBUFS = 4                  # psum pool depth (4 = overlap epilogue with next accum)

# Matmul input dtype for x:
#   "fp16"   rel err ~3e-4, 2-byte DMA (DMA-bound ~27 us)
#   "fp8e3"  rel err ~1.3e-2, 1-byte DMA (PE-bound ~14 us); w stays fp16
# W_E3M4: also quantize w to e3m4 (fallback if mixed-dtype matmul is broken;
# rel err ~1.9e-2)
MM_DTYPE = "fp8e3"
W_E3M4 = False
OUT_FP16 = True             # store y as fp16 (saves 256KB/pass; +~1e-4 rel err)
SX = 2.5                    # x pre-scale before e3m4 cast (max 15.5 = 6.2 sigma)
SW_SCALE = 160.0            # w pre-scale when W_E3M4

_DEC_LO = np.array([-0.010597401784997278, 0.032883011666982945, 0.030841381835986965,
                    -0.18703481171888114, -0.02798376941698385, 0.6308807679295904,
                    0.7148465705525415, 0.23037781330885523], dtype=np.float64)
_DEC_HI = np.array([-0.23037781330885523, 0.7148465705525415, -0.6308807679295904,
                    -0.02798376941698385, 0.18703481171888114, 0.030841381835986965,
                    -0.032883011666982945, -0.010597401784997278], dtype=np.float64)
_H2 = np.stack([_DEC_LO[::-1], _DEC_HI[::-1]])  # [2, 8] correlation filters


def _dwt_level_mat(x):
    """One analysis level (mode='reflect') applied to rows of x [M, N]."""
    n = x.shape[-1]
    l = _H2.shape[-1]
    outsize = (n + l - 1) // 2
    p = 2 * (outsize - 1) - n + l
    if p % 2 == 1:
        x = np.pad(x, ((0, 0), (0, 1)))
    x = np.pad(x, ((0, 0), (p // 2, p // 2)), mode='reflect')
    lo = np.empty((x.shape[0], outsize))
    hi = np.empty((x.shape[0], outsize))
    for o in range(outsize):
        seg = x[:, 2 * o:2 * o + l]
        lo[:, o] = seg @ _H2[0]
        hi[:, o] = seg @ _H2[1]
    return lo, hi


def _dwt_matrix():
    """[64, 84] matrix M with coeffs(v) = v @ M (order: lo3, hi1, hi2, hi3)."""
    lo, highs = np.eye(SWS), []
    for _ in range(3):
        lo, hi = _dwt_level_mat(lo)
        highs.append(hi)
    return np.concatenate([lo] + highs, axis=-1)  # float64 [64, 84]


_DWTM = _dwt_matrix()

_NC_CACHE = {}


def _x_dt():
    return {"fp16": mybir.dt.float16,
            "fp8e3": mybir.dt.float8e3}[MM_DTYPE]


def _np_x_dt():
    return {"fp16": np.float16,
            "fp8e3": ml_dtypes.float8_e3m4}[MM_DTYPE]


def _w_dt():
    if MM_DTYPE == "fp8e3" and W_E3M4:
        return mybir.dt.float8e3
    return mybir.dt.float16


def _np_w_dt():
    if MM_DTYPE == "fp8e3" and W_E3M4:
        return ml_dtypes.float8_e3m4
    return np.float16


def build_nc(reps=1, loop_n=0):
    """Build + compile the per-core Bass module (shared SPMD NEFF).

    reps > 1 unrolls the whole computation `reps` times inside one NEFF;
    loop_n > 0 additionally wraps those reps in a For_i hardware loop.
    Both are only used for benchmarking (amortize host/tunnel dispatch
    overhead); the graded path uses reps=1, loop_n=0.
    """
    key = (MM_DTYPE, W_E3M4, OUT_FP16, GROUP, XBUFS, PSBUFS, reps, loop_n)
    if key in _NC_CACHE:
        return _NC_CACHE[key]
    dt_x = _x_dt()
    dt_w = _w_dt()
    ng = CH // GROUP
    # undo the host-side quantization pre-scales in the epilogue; with fp16
    # weights 1/SX is folded into w on the host instead (exact in fp16)
    descale = 1.0
    if MM_DTYPE == "fp8e3" and W_E3M4:
        descale = 1.0 / (SX * SW_SCALE)
    nc = bacc.Bacc("TRN2", target_bir_lowering=False, debug=False,
                   num_devices=N_CORES)

    xt_dram = nc.dram_tensor("xt", [ng, 128, GROUP * B_LOCAL], dt_x,
                             kind="ExternalInput")
    w_dram = nc.dram_tensor("w", [128, CH, 128], dt_w, kind="ExternalInput")
    b_dram = nc.dram_tensor("b", [128, 1], mybir.dt.float32, kind="ExternalInput")
    dt_out = mybir.dt.float16 if OUT_FP16 else mybir.dt.float32
    y_dram = nc.dram_tensor("y", [128, B_LOCAL], dt_out, kind="ExternalOutput")

    with tile.TileContext(nc) as tc:
        with (
            tc.tile_pool(name="w", bufs=1) as wpool,
            tc.tile_pool(name="x", bufs=XBUFS) as xpool,
            tc.tile_pool(name="o", bufs=2) as opool,
            tc.tile_pool(name="ps", bufs=PSBUFS, space="PSUM") as pspool,
        ):
            w_all = wpool.tile([128, CH, 128], dt_w)
            nc.sync.dma_start(w_all[:], w_dram.ap())
            bias = wpool.tile([128, 1], mybir.dt.float32)
            nc.sync.dma_start(bias[:], b_dram.ap())

            def body():
                # All 32 matmuls of a psum split run back-to-back into the
                # SAME psum bank (bank cycling between consecutive MMs
                # triggers PE micro-idles / HAM oscillation); the whole rep's
                # x (4 MiB) sits in SBUF so the second split re-reads it.
                for _rep in range(reps):
                    psums = [pspool.tile([128, NFREE], mybir.dt.float32,
                                         name=f"psum{i}") for i in range(NSPLIT)]
                    xts = []
                    for g in range(ng):
                        xt = xpool.tile([128, GROUP, B_LOCAL], dt_x)
                        eng = nc.sync if g % 2 == 0 else nc.scalar
                        eng.dma_start(xt[:], xt_dram.ap()[g])
                        xts.append(xt)
                    out = opool.tile([128, B_LOCAL], dt_out)
                    for i in range(NSPLIT):
                        for g in range(ng):
                            for sub in range(GROUP):
                                c = g * GROUP + sub
                                nc.tensor.matmul(
                                    psums[i][:], w_all[:, c, :],
                                    xts[g][:, sub, i * NFREE:(i + 1) * NFREE],
                                    start=(c == 0), stop=(c == CH - 1))
                        nc.scalar.activation(out[:, i * NFREE:(i + 1) * NFREE],
                                             psums[i][:],
                                             mybir.ActivationFunctionType.Lrelu,
                                             bias=bias[:], scale=descale,
                                             alpha=0.01)
                    nc.gpsimd.dma_start(y_dram.ap(), out[:])

            if loop_n > 0:
                with tc.For_i(0, loop_n, 1):
                    body()
            else:
                body()

    nc.compile()
    _NC_CACHE[key] = nc
    return nc


def fold_weights(conv_weight):
    """conv_weight [4, 128, 84, 8, 8] -> W2 [4, 4096, 128] fp32 (fp64 fold)."""
    w2 = np.empty((SW, JDIM, K), dtype=np.float32)
    for s in range(SW):
        # [K, 84, 8, 8] -> [84, 8, 8, K] -> [84, 64*K]
        cws = np.ascontiguousarray(
            conv_weight[s].transpose(1, 2, 3, 0).astype(np.float64)
        ).reshape(84, HWD * HWD * K)
        # [64, 84] @ [84, 64*K] -> [64, (h, w, K)] -> [(q, h, w), K]
        w2[s] = (_DWTM @ cws).reshape(JDIM, K).astype(np.float32)
    return w2


def make_in_maps(x, conv_weight, conv_bias):
    np_x = _np_x_dt()
    np_w = _np_w_dt()
    w2 = fold_weights(conv_weight)
    xr = np.ascontiguousarray(x).reshape(B, SW, JDIM)
    in_maps = []
    for core in range(N_CORES):
        s, half = divmod(core, 2)
        xs = xr[half * B_LOCAL:(half + 1) * B_LOCAL, s, :]  # [1024, 4096]
        xsT = xs.T.astype(np.float32)
        ws = w2[s].astype(np.float64)
        if MM_DTYPE == "fp8e3":
            xsT = np.clip(xsT * SX, -15.5, 15.5)
            if W_E3M4:
                ws = np.clip(ws * SW_SCALE, -15.5, 15.5)
            else:
                ws = ws / SX
        # [(g, sub, p), b] -> [g, p, (sub, b)] so each grouped DMA reads
        # GROUP*B_LOCAL contiguous elements per partition
        xt = np.ascontiguousarray(
            xsT.astype(np_x).reshape(CH // GROUP, GROUP, 128, B_LOCAL)
            .transpose(0, 2, 1, 3)).reshape(CH // GROUP, 128, GROUP * B_LOCAL)
        # w partition-major: [128, CH, 128]; w2[s] is [(c, p), k]
        wt = np.ascontiguousarray(
            ws.astype(np_w).reshape(CH, 128, K).transpose(1, 0, 2))
        bt = np.ascontiguousarray(conv_bias[s].astype(np.float32))[:, None]
        in_maps.append({"xt": xt, "w": wt, "b": bt})
    return in_maps


def gather_out(results):
    out = np.empty((B, SW * K), dtype=np.float32)
    for core in range(N_CORES):
        s, half = divmod(core, 2)
        out[half * B_LOCAL:(half + 1) * B_LOCAL, s * K:(s + 1) * K] = \
            results[core]["y"].T.astype(np.float32)
    return out


def kernel(x, conv_weight, conv_bias):
    nc = build_nc()
    in_maps = make_in_maps(np.asarray(x), np.asarray(conv_weight),
                           np.asarray(conv_bias))
    res = bass_utils.run_bass_kernel_spmd(nc, in_maps,
                                          core_ids=list(range(N_CORES)))
    return gather_out(res.results)
